# revision 30
# baseline (speedup 1.0000x reference)
"""BNN-KDE ELBO kernel for Trainium2, data-parallel over the 8192 samples on 8 cores.

Math (matches the jax reference):
  out = data_lp - kl_term
  data_lp = mean_n sum_b [ -0.5*B*(y_pred[n,b]-y[b])^2 + 0.5*(log B - log 2pi) ]
  kl_term = mean_n [ logsumexp_k comp_lp[n,k] - log K - prior_lp[n] ]
  comp_lp[n,k] = -0.5*(D*log2pi + D*log var[k] + ||w_n - e_k||^2 / var[k])

Device work per core (1024 samples):
  - comp_lp via one PE matmul with contract dim 15:
      lhsT = [w (13 rows); ||w||^2; 1],  rhs = [e/var (13); -0.5/var; colk]
  - exp(comp_lp - m[n]) on ACT with per-partition bias and fused row-sum.
    m[n] = comp_lp[n, rand_idxs[n]] (host-computed; a valid logsumexp shift
    since it is <= the true row max and within 0.5*||eps_n||^2 of it).
  - tiny MLP y_pred: ACT tanh with per-partition scale/bias + fused DVE ops;
    sum_b (y_pred-y)^2 recovered from scalar_tensor_tensor / affine_mul_reduce
    accumulators on host.
Host: O(N*D) prep (gather, transposes), final scalar combine of per-core sums.
"""

import os
import sys

import numpy as np
import ml_dtypes
ml_bf16 = ml_dtypes.bfloat16

for _p in ("/opt/trn_rl_repo",):
    if _p not in sys.path and os.path.isdir(_p):
        sys.path.insert(0, _p)

NUM_NODES = 2
ALPHA = 1.0
BETA = 5.0
KL_BETA = 1.0
LOG_2PI = float(np.log(2.0 * np.pi))

K_COMP = 8192
N_SAMP = 8192
B_X = 2048
D_W = 13

N_CORES = 8
N_LOC = N_SAMP // N_CORES          # 1024 samples per core
P = 128                             # partitions
TILES = N_LOC // P                  # 8 sample-tiles per core
KCHUNK = 2048                       # psum-resident comp_lp chunk (4 banks)
NCHUNK = K_COMP // KCHUNK           # 4 chunks per sample-tile
KSUB = 512                          # fp32 matmul free-dim limit

# pcol column indices (13 weight cols as in reference layout, then -m)
_C_W10, _C_W11, _C_B10, _C_B11 = 0, 1, 2, 3
_C_W200, _C_W201, _C_W210, _C_W211 = 4, 5, 6, 7
_C_B20, _C_B21, _C_W30, _C_W31, _C_B3 = 8, 9, 10, 11, 12
_C_NEGM = 13
PCOLS = 14

_PROG = None
LAST_EXEC_NS = None


def build_program():
    import concourse.bass as bass
    import concourse.tile as tile
    from concourse import bacc, mybir

    f32 = mybir.dt.float32
    f32r = mybir.dt.float32r
    bf16 = mybir.dt.bfloat16
    Alu = mybir.AluOpType
    Act = mybir.ActivationFunctionType

    nc = bacc.Bacc("TRN2", target_bir_lowering=False, debug=False,
                   num_devices=N_CORES)

    empT_d = nc.declare_dram_parameter("empT", [15, K_COMP], f32r, isOutput=False)
    wT_d = nc.declare_dram_parameter("wT", [15, N_LOC], f32r, isOutput=False)
    pcol_d = nc.declare_dram_parameter("pcol", [N_LOC, PCOLS], f32, isOutput=False)
    xv_d = nc.declare_dram_parameter("xv", [B_X], bf16, isOutput=False)
    nyv_d = nc.declare_dram_parameter("nyv", [B_X], f32, isOutput=False)
    qparts_d = nc.declare_dram_parameter("qparts", [P, TILES * NCHUNK + KCHUNK // KSUB - 1], f32, isOutput=True)
    sv2_d = nc.declare_dram_parameter("sv2", [P, TILES], f32, isOutput=True)
    samr_d = nc.declare_dram_parameter("samr", [P, TILES], f32, isOutput=True)

    with tile.TileContext(nc) as tc:
        with (
            tc.tile_pool(name="const", bufs=1) as cpool,
            tc.tile_pool(name="hpool", bufs=3) as hpool,
            tc.tile_pool(name="h2pool", bufs=4) as h2pool,
            tc.tile_pool(name="vpool", bufs=2) as wpool,
            tc.tile_pool(name="spool", bufs=2) as spool,
            tc.tile_pool(name="dump", bufs=1) as dpool,
            tc.tile_pool(name="psum", bufs=2, space=bass.MemorySpace.PSUM) as ppool,
        ):
            empT = cpool.tile([15, K_COMP], f32r)
            nc.sync.dma_start(empT[:], empT_d[:])
            wT = cpool.tile([15, N_LOC], f32r)
            nc.sync.dma_start(wT[:], wT_d[:])
            warm = cpool.tile([P, 1], f32)
            nc.vector.memset(warm[:], 0.0)
            nc.scalar.activation(warm[:], warm[:], Act.Exp)

            pcs = []
            for t in range(TILES):
                pc = cpool.tile([P, PCOLS], f32, tag=f"pc{t}")
                nc.sync.dma_start(pc[:], pcol_d[t * P:(t + 1) * P, :])
                pcs.append(pc)

            xb = cpool.tile([P, B_X], bf16)
            nc.sync.dma_start(xb[:], xv_d[:].partition_broadcast(P))
            nyb = cpool.tile([P, B_X], f32)
            nc.sync.dma_start(nyb[:], nyv_d[:].partition_broadcast(P))

            qparts_sb = cpool.tile([P, TILES * NCHUNK + KCHUNK // KSUB - 1], f32)
            sv2_sb = cpool.tile([P, TILES], f32)
            samr_sb = cpool.tile([P, TILES], f32)

            def emit_mlp(t):
                pc = pcs[t]
                # ---- MLP block ----
                h0 = hpool.tile([P, B_X], bf16, tag="h0")
                nc.scalar.activation(h0[:], xb[:], Act.Tanh,
                                     bias=pc[:, _C_B10:_C_B10 + 1],
                                     scale=pc[:, _C_W10:_C_W10 + 1])
                h1 = hpool.tile([P, B_X], bf16, tag="h1")
                nc.scalar.activation(h1[:], xb[:], Act.Tanh,
                                     bias=pc[:, _C_B11:_C_B11 + 1],
                                     scale=pc[:, _C_W11:_C_W11 + 1])

                t0 = spool.tile([P, B_X], bf16, tag="t01")
                nc.vector.tensor_scalar(t0[:], h1[:],
                                        pc[:, _C_W201:_C_W201 + 1],
                                        pc[:, _C_B20:_C_B20 + 1],
                                        Alu.mult, Alu.add)
                p0 = spool.tile([P, B_X], bf16, tag="p01")
                nc.vector.tensor_scalar(p0[:], h0[:],
                                        pc[:, _C_W200:_C_W200 + 1], None,
                                        Alu.mult)
                r01 = spool.tile([P, 2 * B_X], bf16, tag="r01")
                nc.vector.tensor_tensor(r01[:, :B_X], p0[:], t0[:], Alu.add)
                t1 = spool.tile([P, B_X], bf16, tag="t01")
                nc.vector.tensor_scalar(t1[:], h1[:],
                                        pc[:, _C_W211:_C_W211 + 1],
                                        pc[:, _C_B21:_C_B21 + 1],
                                        Alu.mult, Alu.add)
                p1 = spool.tile([P, B_X], bf16, tag="p01")
                nc.vector.tensor_scalar(p1[:], h0[:],
                                        pc[:, _C_W210:_C_W210 + 1], None,
                                        Alu.mult)
                nc.vector.tensor_tensor(r01[:, B_X:], p1[:], t1[:], Alu.add)
                h2 = h2pool.tile([P, 2 * B_X], bf16, tag="h2")
                nc.scalar.activation(h2[:], r01[:], Act.Tanh)

                # v = w3_1*h2_1 - y ; v2 = w3_0*h2_0 + v = y_pred - y - b3
                v = spool.tile([P, B_X], f32, tag="v")
                nc.vector.scalar_tensor_tensor(v[:], h2[:, B_X:],
                                               pc[:, _C_W31:_C_W31 + 1],
                                               nyb[:], Alu.mult, Alu.add)
                v2 = wpool.tile([P, B_X], f32, tag="v2")
                nc.vector.scalar_tensor_tensor(v2[:], h2[:, :B_X],
                                               pc[:, _C_W30:_C_W30 + 1],
                                               v[:], Alu.mult, Alu.add,
                                               accum_out=sv2_sb[:, t:t + 1])
                # samr = sum (v2 + b3) * v2
                zdump = dpool.tile([P, B_X], f32, tag="zdump")
                nc.vector.affine_mul_reduce(zdump[:], samr_sb[:, t:t + 1],
                                            v2[:], v2[:],
                                            scale=1.0,
                                            bias=pc[:, _C_B3:_C_B3 + 1])

            def emit_kde(t):
                pc = pcs[t]
                lhsT = wT[:, t * P:(t + 1) * P]
                # ---- KDE block: comp_lp -> exp(. - m) -> row sums ----
                # Tile 0 chunk 0 runs exp per 512-wide matmul so ACT starts
                # ~3us earlier instead of waiting on 4 cold serial matmuls.
                for c in range(NCHUNK):
                    ps = ppool.tile([P, KCHUNK], f32, tag="ps")
                    sub = (t == 0 and c == 0)
                    for s in range(KCHUNK // KSUB):
                        k0 = c * KCHUNK + s * KSUB
                        nc.tensor.matmul(
                            ps[:, s * KSUB:(s + 1) * KSUB],
                            lhsT,
                            empT[:, k0:k0 + KSUB],
                            start=True, stop=True,
                        )
                        if sub:
                            qcol = 0 if s == 0 else TILES * NCHUNK + s - 1
                            nc.scalar.activation(
                                ps[:, s * KSUB:(s + 1) * KSUB],
                                ps[:, s * KSUB:(s + 1) * KSUB], Act.Exp,
                                bias=pc[:, _C_NEGM:_C_NEGM + 1], scale=1.0,
                                accum_out=qparts_sb[:, qcol:qcol + 1],
                            )
                    if not sub:
                        nc.scalar.activation(
                            ps[:], ps[:], Act.Exp,
                            bias=pc[:, _C_NEGM:_C_NEGM + 1], scale=1.0,
                            accum_out=qparts_sb[:, t * NCHUNK + c:t * NCHUNK + c + 1],
                        )

            # Tile-0 KDE first (its inputs land earliest: no broadcast-DMA
            # dependency), then every MLP block, then the remaining KDE
            # blocks: the trailing ~60us of ACT exp work has no DVE
            # dependents, so the DVE tail fully overlaps, and the scheduler
            # backfills any ACT idle slots with ready exp chunks.
            emit_kde(0)
            for t in range(TILES):
                emit_mlp(t)
            for t in range(1, TILES):
                emit_kde(t)

            nc.sync.dma_start(qparts_d[:], qparts_sb[:])
            nc.sync.dma_start(sv2_d[:], sv2_sb[:])
            nc.sync.dma_start(samr_d[:], samr_sb[:])

    nc.compile()
    return nc


def _get_prog():
    global _PROG
    if _PROG is None:
        _PROG = build_program()
    return _PROG


def host_prep(emp_samples, log_kde_rhos, x, y, eps, rand_idxs):
    """Returns (per-core in_maps, host-side combine context)."""
    emp = np.asarray(emp_samples, np.float32)
    logr = np.asarray(log_kde_rhos, np.float32)
    x = np.asarray(x, np.float32).reshape(-1)
    y = np.asarray(y, np.float32).reshape(-1)
    eps = np.asarray(eps, np.float32)
    idx = np.asarray(rand_idxs).astype(np.int64)

    # softplus in f32, matching jax.nn.softplus
    kde_std = np.logaddexp(np.float32(0.0), logr).astype(np.float32)
    kde_var = (kde_std * kde_std).astype(np.float32)

    esq = np.einsum("kd,kd->k", emp, emp, dtype=np.float32).astype(np.float32)
    colconst = (-0.5 * (D_W * LOG_2PI + D_W * np.log(kde_var))).astype(np.float32)
    a = (-0.5 / kde_var).astype(np.float32)

    # empT rows: e/var (13), a, colconst + a*esq
    empT = np.empty((15, K_COMP), np.float32)
    empT[:D_W] = (emp / kde_var[:, None]).T
    empT[D_W] = a
    empT[D_W + 1] = colconst + a * esq

    # per-sample things
    std_g = kde_std[idx]
    w = (emp[idx] + eps * std_g[:, None]).astype(np.float32)
    wsq = np.einsum("nd,nd->n", w, w, dtype=np.float32).astype(np.float32)
    epssq = np.einsum("nd,nd->n", eps, eps, dtype=np.float32)
    m = (colconst[idx] - 0.5 * epssq).astype(np.float32)

    in_maps = []
    for c in range(N_CORES):
        sl = slice(c * N_LOC, (c + 1) * N_LOC)
        wT = np.empty((15, N_LOC), np.float32)
        wT[:D_W] = w[sl].T
        wT[D_W] = wsq[sl]
        wT[D_W + 1] = 1.0
        pcol = np.empty((N_LOC, PCOLS), np.float32)
        pcol[:, :D_W] = w[sl]
        pcol[:, _C_NEGM] = -m[sl]
        in_maps.append({
            "empT": np.ascontiguousarray(empT),
            "wT": np.ascontiguousarray(wT),
            "pcol": np.ascontiguousarray(pcol),
            "xv": x.astype(ml_bf16),
            "nyv": np.ascontiguousarray(-y),
        })

    ctx = {"w": w, "wsq": wsq, "m": m, "b3": w[:, _C_B3], "y": y}
    return in_maps, ctx


def host_combine(ctx, qsum, sv2, samr):
    """qsum/sv2/samr are full [N_SAMP] float64 arrays gathered from cores."""
    m = ctx["m"].astype(np.float64)
    wsq = ctx["wsq"].astype(np.float64)
    b3 = ctx["b3"].astype(np.float64)
    y = ctx["y"].astype(np.float64)

    q_lp = m + np.log(qsum) - np.log(float(K_COMP))
    prior_lp = -0.5 * ALPHA * wsq + D_W * 0.5 * (np.log(ALPHA) - LOG_2PI)
    kl_term = np.mean(q_lp - prior_lp)

    ssq = samr + b3 * sv2 + B_X * b3 * b3   # sum_b (y_pred - y)^2 per sample
    data_lp = (-0.5 * BETA) * np.mean(ssq) + B_X * 0.5 * (np.log(BETA) - LOG_2PI)
    return np.float32(data_lp - KL_BETA * kl_term)


def kernel(emp_samples, log_kde_rhos, x, y, eps, rand_idxs):
    global LAST_EXEC_NS
    from concourse.bass_utils import run_bass_kernel_spmd

    nc = _get_prog()
    in_maps, ctx = host_prep(emp_samples, log_kde_rhos, x, y, eps, rand_idxs)

    trace = bool(int(os.environ.get("BNN_TRACE", "0")))
    try:
        res = run_bass_kernel_spmd(nc, in_maps, core_ids=list(range(N_CORES)),
                                   trace=trace)
    except ModuleNotFoundError:
        # NTFF profile hook unavailable in this container; run untraced.
        res = run_bass_kernel_spmd(nc, in_maps, core_ids=list(range(N_CORES)))
    LAST_EXEC_NS = res.exec_time_ns

    def _qsum(arr):
        arr = arr.astype(np.float64)
        main = arr[:, :TILES * NCHUNK].reshape(P, TILES, NCHUNK).sum(axis=2)
        main[:, 0] += arr[:, TILES * NCHUNK:].sum(axis=1)
        return main.T.reshape(N_LOC)

    qsum = np.concatenate([_qsum(r["qparts"]) for r in res.results])
    sv2 = np.concatenate(
        [r["sv2"].astype(np.float64).T.reshape(N_LOC) for r in res.results])
    samr = np.concatenate(
        [r["samr"].astype(np.float64).T.reshape(N_LOC) for r in res.results])
    return host_combine(ctx, qsum, sv2, samr)
